# revision 57
# baseline (speedup 1.0000x reference)
"""Trainium2 Bass kernel for RoPE + GQA causal attention (B=1, S=2048, HID=2048,
NH=16, NKV=4, HD=128), tensor-parallel over heads across 8 NeuronCores.

Core c computes q heads {2c, 2c+1} and kv head c//2 plus the matching wo
input-dim slice; each core emits a partial [S, HID] (f16) output and the host
sums the 8 partials.

v2 design notes (vs the 268us baseline):
  - 1024-wide bf16 matmul streams everywhere (proj, scores, attnV, rope raw),
    paired-head PSUM tiles [128, 2, 512] spanning 2 banks.
  - softmax denominator off the PE: bf16 pairwise tree-sum of the exp tiles on
    DVE + one tiny ones-matmul per (chunk, head); reciprocal on DVE,
    partition-broadcast on Pool.
  - causal masking by multiplying the exp tile with a 0/1 triangle (DVE),
    zeroing fully-masked columns with Pool memsets; no PSUM mask adds.
  - exp: ONE activation per k-tile covering both heads; ACT runs Exp/Copy only
    (single activation table, no table reloads).
  - phase interleave: proj chunk 0 -> attn cols [0,1024) -> proj chunk 1 ->
    attn cols [1024,2048); out-projection units of chunk j interleave into the
    PE stream of later chunks so the PE never idles (HAM stays at 2.4 GHz).
  - V transposed to [s,d] tiles via DMA-transpose (no PE/psum involved).
  - output partial written as f16 (half the DMA), copies split ACT/DVE.
"""

import os
import sys
from contextlib import ExitStack

for _p in ("/opt/trn_rl_repo", "/root/.axon_site/_ro/trn_rl_repo"):
    if os.path.isdir(_p) and _p not in sys.path:
        sys.path.append(_p)

import ml_dtypes
import numpy as np

import concourse.bass as bass
import concourse.mybir as mybir
import concourse.tile as tile
from concourse import bacc, bass_utils

S, HID, NH, NKV, HD = 2048, 2048, 16, 4, 128
HH = HD // 2  # 64
NCORES = 8
QH = NH // NCORES  # 2 q heads per core
SCALE = float(1.0 / np.sqrt(HD))

F32 = mybir.dt.float32
BF16 = mybir.dt.bfloat16
F16 = mybir.dt.float16
F8 = mybir.dt.float8e4
NPBF = ml_dtypes.bfloat16
NPF8 = ml_dtypes.float8_e4m3fn

# q/k projections run in fp8 (DoubleRow); weights are pre-scaled by W8S on the
# host, so scores carry W8S^2 and the exp scale divides it back out
W8S = 32.0

NKC = HID // 128   # 16 contraction chunks
NK2 = HID // 256   # 8 double-row contraction chunks
SC1 = 1024         # phase-1 s-chunk
NC1 = S // SC1     # 2
# phase-2 attention chunks: (col0, width); tail kept short (last chunk 128)
ATTN_CHUNKS = [(0, 512), (512, 512), (1024, 512), (1536, 384), (1920, 128)]


def build_graph():
    nc = bacc.Bacc(trn_type="TRN2", enable_partition_id=False)

    xT = nc.dram_tensor("xt", [HID, S], BF16, kind="ExternalInput")
    x8d = nc.dram_tensor("x8", [HID, S], F8, kind="ExternalInput")
    wvT = nc.dram_tensor("wvt", [HID, HD], BF16, kind="ExternalInput")
    wqk8d = nc.dram_tensor("wqk8", [HID, 3 * HD], F8, kind="ExternalInput")
    woT = nc.dram_tensor("wot", [QH * HD, HID], BF16, kind="ExternalInput")
    c1d = nc.dram_tensor("c1", [HD, S], BF16, kind="ExternalInput")
    c2d = nc.dram_tensor("c2", [HD, S], BF16, kind="ExternalInput")
    r1d = nc.dram_tensor("r1t", [HD, HD], BF16, kind="ExternalInput")
    r2d = nc.dram_tensor("r2t", [HD, HD], BF16, kind="ExternalInput")
    outd = nc.dram_tensor("out", [S, HID], F16, kind="ExternalOutput")

    xT_t = xT.rearrange("(ko p) s -> p ko s", p=128)       # [128, 16, 2048]
    x8_t = x8d.rearrange("(k2 two p) s -> p k2 two s", p=128, two=2)
    wv_t = wvT.rearrange("(ko p) o -> p ko o", p=128)      # [128, 16, 128]
    wqk8_t = wqk8d.rearrange("(k2 two p) o -> p k2 two o", p=128, two=2)
    wo_t = woT.rearrange("(g p) h -> p g h", p=128)        # [128, 2, 2048]

    with tile.TileContext(nc) as tc, ExitStack() as ctx:
        # ---------- pools (all open for the whole kernel; phases interleave)
        consts = ctx.enter_context(tc.tile_pool(name="consts", bufs=1))
        persist = ctx.enter_context(tc.tile_pool(name="persist", bufs=1))
        xtp = ctx.enter_context(tc.tile_pool(name="xtp", bufs=1))
        x8p = ctx.enter_context(tc.tile_pool(name="x8p", bufs=1))
        rawp = ctx.enter_context(tc.tile_pool(name="rawp", bufs=4))
        t12p = ctx.enter_context(tc.tile_pool(name="t12p", bufs=2))
        ep = ctx.enter_context(tc.tile_pool(name="ep", bufs=16))
        trp = ctx.enter_context(tc.tile_pool(name="trp", bufs=6))
        obp = ctx.enter_context(tc.tile_pool(name="obp", bufs=6))
        dbp = ctx.enter_context(tc.tile_pool(name="dbp", bufs=2))
        # PSUM: 4 + 2 + 2 = 8 banks exactly
        p1 = ctx.enter_context(
            tc.tile_pool(name="p1", bufs=2, space="PSUM"))   # [128,2,512]f32 x2
        p2 = ctx.enter_context(
            tc.tile_pool(name="p2", bufs=1, space="PSUM"))   # [128,2,512]f32 x1
        p3 = ctx.enter_context(
            tc.tile_pool(name="p3", bufs=2, space="PSUM"))   # [128,512]f32 x2

        # ---------- persistent SBUF
        q_pair = persist.tile([128, QH, S], BF16, tag="q_pair")
        kT = persist.tile([128, S], BF16, tag="kT")
        v_sd = persist.tile([128, S // 128, HD], BF16, tag="v_sd")
        ao_pair = persist.tile([128, QH, S], BF16, tag="ao_pair")
        wv_sb = persist.tile([128, NKC, HD], BF16, tag="wv_sb")
        wqk8_sb = persist.tile([128, NK2, 2, 3 * HD], F8, tag="wqk8_sb")
        wo_sb = persist.tile([128, QH, HID], BF16, tag="wo_sb")
        c1_sb = persist.tile([128, S], BF16, tag="c1_sb")
        c2_sb = persist.tile([128, S], BF16, tag="c2_sb")

        r1_sb = consts.tile([128, 128], BF16)
        r2_sb = consts.tile([128, 128], BF16)
        ones_col = consts.tile([128, 1], BF16)
        nc.vector.memset(ones_col, 1.0)
        # 0/1 lower-triangle keep mask (keep where col >= row), both heads
        tri01 = consts.tile([128, QH, 128], BF16)
        nc.gpsimd.memset(tri01, 1.0)
        nc.gpsimd.affine_select(
            out=tri01,
            in_=tri01,
            compare_op=mybir.AluOpType.is_ge,
            fill=0.0,
            base=0,
            pattern=[[0, QH], [1, 128]],  # value = col
            channel_multiplier=-1,        # - row
        )

        # ---------- initial DMAs (kc-parity interleave so chunks land in order)
        xt0 = xtp.tile([128, NKC, SC1], BF16, tag="xt", name="xt0")
        x80 = x8p.tile([128, NK2, 2, SC1], F8, tag="x8", name="x80")
        for kc in range(NKC):
            eng = nc.sync if kc % 2 == 0 else nc.gpsimd
            eng.dma_start(wv_sb[:, kc, :], wv_t[:, kc, :])
            eng.dma_start(xt0[:, kc, :], xT_t[:, kc, 0:SC1])
            if kc % 2 == 1:
                kc2 = kc // 2
                eng2 = nc.gpsimd if kc % 4 == 1 else nc.sync
                eng2.dma_start(wqk8_sb[:, kc2, :, :], wqk8_t[:, kc2, :, :])
                eng2.dma_start(x80[:, kc2, :, :], x8_t[:, kc2, :, 0:SC1])
        nc.scalar.dma_start(r1_sb, r1d[:, :])
        nc.scalar.dma_start(r2_sb, r2d[:, :])
        nc.scalar.dma_start(c1_sb, c1d[:, :])
        nc.scalar.dma_start(c2_sb, c2d[:, :])
        for g in range(QH):
            nc.scalar.dma_start(wo_sb[:, g, :], wo_t[:, g, :])

        # ---------------- out-projection work queue --------------------
        # each unit: 2 accumulating matmuls + psum->f16 copy + DMA out
        op_queue = []
        op_flip = [0]

        def emit_op_unit():
            if not op_queue:
                return False
            st, hc = op_queue.pop(0)
            ssl = slice(st * 128, (st + 1) * 128)
            hsl = slice(hc * 512, (hc + 1) * 512)
            ps_o = p3.tile([128, 512], F32, tag="oc", name="ps_o")
            nc.tensor.matmul(ps_o, ao_pair[:, 0, ssl], wo_sb[:, 0, hsl],
                             start=True, stop=False)
            nc.tensor.matmul(ps_o, ao_pair[:, 1, ssl], wo_sb[:, 1, hsl],
                             start=False, stop=True)
            ob = obp.tile([128, 512], F16, tag="ob", name="ob")
            if op_flip[0] % 8 < 5:
                nc.scalar.copy(out=ob, in_=ps_o)
            else:
                nc.vector.tensor_copy(out=ob, in_=ps_o)
            nc.sync.dma_start(outd[ssl, hsl], ob)
            op_flip[0] += 1
            return True

        def queue_op_chunk(c0, w):
            for st in range(c0 // 128, (c0 + w) // 128):
                for hc in range(4):
                    op_queue.append((st, hc))

        # ---------------- phase 1: projections + RoPE ------------------
        def rope_half(t, j1, hh, ps_uw):
            """DVE combine for rope half hh of target t (0,1=q heads, 2=k)."""
            ps_u, ps_w = ps_uw
            base = j1 * SC1 + hh * 512
            csl = slice(base, base + 512)
            t1 = t12p.tile([128, 512], F32, tag="t1", name="t1")
            t2 = t12p.tile([128, 512], F32, tag="t2", name="t2")
            nc.vector.tensor_mul(out=t1, in0=ps_u, in1=c1_sb[:, csl])
            nc.vector.tensor_mul(out=t2, in0=ps_w, in1=c2_sb[:, csl])
            dst = q_pair[:, t, csl] if t < 2 else kT[:, csl]
            nc.vector.tensor_add(out=dst, in0=t1, in1=t2)

        def p1_chunk(j1, xt, xt8, fin_prev=None):
            # pending rope matmul sub-steps: (t, hh, raw)
            rope_subs = []
            sub_i = [0]

            def emit_rope_sub():
                if not rope_subs:
                    return
                t, hh, raw = rope_subs.pop(0)
                ps_uw = p2.tile([128, 2, 512], F32, tag="uw", name="ps_uw")
                sub_i[0] += 1
                nc.tensor.matmul(ps_uw[:, 0, :], r1_sb, raw[:, hh, :],
                                 start=True, stop=True)
                nc.tensor.matmul(ps_uw[:, 1, :], r2_sb, raw[:, hh, :],
                                 start=True, stop=True)
                rope_half(t, j1, hh, (ps_uw[:, 0, :], ps_uw[:, 1, :]))

            DR = mybir.MatmulPerfMode.DoubleRow

            def dr_mm(ps, kc2, osl, start, stop):
                """fp8 DoubleRow projection matmul pair (both s-halves)."""
                nc.tensor.matmul(
                    ps[:, 0, :], wqk8_sb[:, kc2, :, osl],
                    xt8[:, kc2, :, 0:512],
                    start=start, stop=stop, perf_mode=DR,
                )
                nc.tensor.matmul(
                    ps[:, 1, :], wqk8_sb[:, kc2, :, osl],
                    xt8[:, kc2, :, 512:SC1],
                    start=start, stop=stop, perf_mode=DR,
                )

            def hook_a(kc):
                if kc == 4 and fin_prev is not None:
                    fin_prev()
                elif kc in (4, 8, 12):
                    emit_op_unit()
                    emit_op_unit()

            # pass A: v (bf16, DMA-paced) + k (fp8 double-row) together
            pa = p1.tile([128, 2, 512], F32, tag="mm", name="pa")
            pb = p1.tile([128, 2, 512], F32, tag="mm", name="pb")
            for kc in range(NKC):
                hook_a(kc)
                nc.tensor.matmul(
                    pa[:, 0, :], wv_sb[:, kc, :], xt[:, kc, 0:512],
                    start=(kc == 0), stop=(kc == NKC - 1),
                )
                nc.tensor.matmul(
                    pa[:, 1, :], wv_sb[:, kc, :], xt[:, kc, 512:SC1],
                    start=(kc == 0), stop=(kc == NKC - 1),
                )
                if kc % 2 == 1:
                    kc2 = kc // 2
                    dr_mm(pb, kc2, slice(256, 384),
                          start=(kc2 == 0), stop=(kc2 == NK2 - 1))
            raw_v = rawp.tile([128, 2, 512], BF16, tag="raw", name="raw_v")
            nc.scalar.copy(out=raw_v, in_=pa)
            raw_k = rawp.tile([128, 2, 512], BF16, tag="raw", name="raw_k")
            nc.scalar.copy(out=raw_k, in_=pb)
            for b in range(SC1 // 128):
                st = j1 * (SC1 // 128) + b
                vr2 = raw_v[:, b // 4, (b % 4) * 128:(b % 4 + 1) * 128]
                eng = nc.sync if b % 2 == 0 else nc.scalar
                eng.dma_start_transpose(v_sd[:, st, :], vr2)
            rope_subs.append((2, 0, raw_k))
            rope_subs.append((2, 1, raw_k))
            # pass B: q0 + q1 fp8 double-row (k rope drains inside)
            pq0 = p1.tile([128, 2, 512], F32, tag="mm", name="pq0")
            pq1 = p1.tile([128, 2, 512], F32, tag="mm", name="pq1")
            for kc2 in range(NK2):
                if kc2 in (2, 4, 6):
                    if rope_subs:
                        emit_rope_sub()
                    else:
                        emit_op_unit()
                        emit_op_unit()
                dr_mm(pq0, kc2, slice(0, 128),
                      start=(kc2 == 0), stop=(kc2 == NK2 - 1))
                dr_mm(pq1, kc2, slice(128, 256),
                      start=(kc2 == 0), stop=(kc2 == NK2 - 1))
            raw_q0 = rawp.tile([128, 2, 512], BF16, tag="raw", name="raw_q0")
            nc.scalar.copy(out=raw_q0, in_=pq0)
            raw_q1 = rawp.tile([128, 2, 512], BF16, tag="raw", name="raw_q1")
            nc.scalar.copy(out=raw_q1, in_=pq1)
            rope_subs.append((0, 0, raw_q0))
            rope_subs.append((1, 0, raw_q1))
            rope_subs.append((0, 1, raw_q0))
            rope_subs.append((1, 1, raw_q1))
            while rope_subs:
                emit_rope_sub()
                emit_op_unit()
                emit_op_unit()
                emit_op_unit()

        # ---------------- phase 2: attention chunk ---------------------
        # Returns a finalize closure (den -> recip -> broadcast -> normalize
        # -> queue out-projection) that the CALLER emits later, from inside
        # the next PE-busy region, so chunk boundaries never stall the PE.
        def attn_chunk(c0, w, fin_prev=None):
            nk = (c0 + w) // 128
            # tree state: list of (level, tile) for the den pairwise sum
            tree = []

            def tree_add(entry):
                tree.append(entry)
                while len(tree) >= 2 and tree[-1][0] == tree[-2][0]:
                    l1, a = tree.pop()
                    _, b = tree.pop()
                    s_ = trp.tile([128, QH, 512], BF16, tag="tr", name="tsum")
                    with nc.allow_low_precision("bf16 den tree"):
                        nc.vector.tensor_add(
                            out=s_[:, :, :w], in0=a[:, :, :w], in1=b[:, :, :w])
                    tree.append((l1 + 1, s_))

            pend = []  # (k, v0, e) awaiting attnV
            st = {"ps_av": None}

            def emit_attnv():
                pk, pv0, pe = pend.pop(0)
                if st["ps_av"] is None:
                    st["ps_av"] = p2.tile([128, 2, 512], F32, tag="uw",
                                          name="ps_av")
                for h in range(QH):
                    nc.tensor.matmul(
                        st["ps_av"][:, h, pv0:w], v_sd[:, pk, :],
                        pe[:, h, pv0:w],
                        start=(pk == 0), stop=(pk == nk - 1),
                    )

            for k in range(nk):
                lo = 128 * k - c0  # diag block offset in chunk cols
                v0 = max(lo, 0)
                ps_s = p1.tile([128, 2, 512], F32, tag="mm", name="ps_s")
                for h in range(QH):
                    nc.tensor.matmul(
                        ps_s[:, h, v0:w], kT[:, k * 128:(k + 1) * 128],
                        q_pair[:, h, c0 + v0:c0 + w], start=True, stop=True,
                    )
                e = ep.tile([128, QH, 512], BF16, tag="e", name="e")
                nc.scalar.activation(
                    out=e[:, :, v0:w], in_=ps_s[:, :, v0:w],
                    func=mybir.ActivationFunctionType.Exp,
                    scale=SCALE / (W8S * W8S),
                )
                if v0 > 0:
                    nc.gpsimd.memset(e[:, :, 0:v0], 0.0)
                if lo > -128:  # diagonal tile: mask cols [lo, lo+128)
                    d0, d1 = max(lo, 0), min(lo + 128, w)
                    nc.vector.tensor_mul(
                        out=e[:, :, d0:d1], in0=e[:, :, d0:d1],
                        in1=tri01[:, :, d0 - lo:d1 - lo])
                tree_add((0, e))
                pend.append((k, v0, e))
                if k == 1 and fin_prev is not None:
                    fin_prev()
                if len(pend) > 3:
                    emit_attnv()
                # drain out-projection units, keeping 3 in reserve to cover
                # the next chunk-boundary normalize latency
                if k < 3 or len(op_queue) > 6:
                    emit_op_unit()
            while pend:
                emit_attnv()

            # collapse leftover tree nodes (mixed levels)
            while len(tree) > 1:
                _, a = tree.pop()
                l2, b = tree.pop()
                s_ = trp.tile([128, QH, 512], BF16, tag="tr", name="tsum")
                with nc.allow_low_precision("bf16 den tree"):
                    nc.vector.tensor_add(
                        out=s_[:, :, :w], in0=a[:, :, :w], in1=b[:, :, :w])
                tree.append((l2 + 1, s_))
            acc = tree[0][1]
            ps_av = st["ps_av"]

            def fin():
                # denominator -> reciprocal -> broadcast -> normalize
                rd = dbp.tile([1, QH, 512], F32, tag="rd", name="rd")
                for h in range(QH):
                    psd = p3.tile([128, 512], F32, tag="oc", name="psd")
                    nc.tensor.matmul(psd[0:1, :w], ones_col, acc[:, h, :w],
                                     start=True, stop=True)
                    nc.vector.reciprocal_approx_fast(
                        out=rd[:, h, :w], in_=psd[0:1, :w])
                db = dbp.tile([128, QH, 512], F32, tag="db", name="db")
                nc.gpsimd.partition_broadcast(db[:, :, :w], rd[:1, :, :w])
                nc.vector.tensor_mul(
                    out=ao_pair[:, :, c0:c0 + w], in0=ps_av[:, :, :w],
                    in1=db[:, :, :w])
                queue_op_chunk(c0, w)

            return fin

        # ---------------- emission order -------------------------------
        p1_chunk(0, xt0, x80)
        # prefetch x chunk 1 (bf16 on the ACT hw-dge queue, fp8 on sync)
        xt1 = xtp.tile([128, NKC, SC1], BF16, tag="xt", name="xt1")
        for kc in range(NKC):
            nc.scalar.dma_start(xt1[:, kc, :], xT_t[:, kc, SC1:2 * SC1])
        x81 = x8p.tile([128, NK2, 2, SC1], F8, tag="x8", name="x81")
        for kc2 in range(NK2):
            nc.sync.dma_start(x81[:, kc2, :, :], x8_t[:, kc2, :, SC1:2 * SC1])
        fin0 = attn_chunk(0, 512)
        fin1 = attn_chunk(512, 512, fin_prev=fin0)
        p1_chunk(1, xt1, x81, fin_prev=fin1)
        fin2 = attn_chunk(1024, 512)
        fin3 = attn_chunk(1536, 384, fin_prev=fin2)
        fin4 = attn_chunk(1920, 128, fin_prev=fin3)
        fin4()
        while emit_op_unit():
            pass

    nc.finalize()
    return nc


def shard_inputs(x, cos, sin, wq, wk, wv, wo):
    x = np.asarray(x, np.float32).reshape(S, HID)
    cos = np.asarray(cos, np.float32)
    sin = np.asarray(sin, np.float32)
    wq = np.asarray(wq, np.float32)
    wk = np.asarray(wk, np.float32)
    wv = np.asarray(wv, np.float32)
    wo = np.asarray(wo, np.float32)

    xT = np.ascontiguousarray(x.T).astype(NPBF)

    cos_h, sin_h = cos[:, :HH].T, sin[:, :HH].T       # [64, S]
    c1 = np.ascontiguousarray(
        np.concatenate([cos_h, -sin_h], axis=0)).astype(NPBF)
    c2 = np.ascontiguousarray(
        np.concatenate([sin_h, cos_h], axis=0)).astype(NPBF)

    r1 = np.zeros((HD, HD), np.float32)
    for i in range(HH // 2):
        r1[2 * i, 2 * i + 1] = -1.0
        r1[2 * i + 1, 2 * i] = 1.0
    r1[HH:, :] = r1[:HH, :]
    r2 = np.zeros((HD, HD), np.float32)
    for d in range(HH):
        r2[d, d + HH] = 1.0
        r2[d + HH, d + HH] = 1.0
    r1t = np.ascontiguousarray(r1.T).astype(NPBF)  # lhsT for out = R1 @ rhs
    r2t = np.ascontiguousarray(r2.T).astype(NPBF)

    x8 = np.ascontiguousarray(x.T).astype(NPF8)

    in_maps = []
    for c in range(NCORES):
        h0 = QH * c
        kvh = h0 * NKV // NH
        wq_c = wq[h0 * HD:(h0 + QH) * HD, :]             # [256, HID]
        wk_c = wk[kvh * HD:(kvh + 1) * HD, :]            # [128, HID]
        wv_c = wv[kvh * HD:(kvh + 1) * HD, :]
        wvT_c = np.ascontiguousarray(wv_c.T).astype(NPBF)
        wqk8_c = np.ascontiguousarray(
            (np.concatenate([wq_c, wk_c], axis=0) * W8S).T).astype(NPF8)
        woT_c = np.ascontiguousarray(
            wo[:, h0 * HD:(h0 + QH) * HD].T).astype(NPBF)
        in_maps.append({
            "xt": xT,
            "x8": x8,
            "wvt": wvT_c,
            "wqk8": wqk8_c,
            "wot": woT_c,
            "c1": c1,
            "c2": c2,
            "r1t": r1t,
            "r2t": r2t,
        })
    return in_maps


_CACHED_NC = None


def kernel(x, cos, sin, wq, wk, wv, wo, _trace=False, _tmpdir=None):
    global _CACHED_NC
    in_maps = shard_inputs(x, cos, sin, wq, wk, wv, wo)
    if _CACHED_NC is None:
        _CACHED_NC = build_graph()
    nc = _CACHED_NC
    res = bass_utils.run_bass_kernel_spmd(
        nc, in_maps, core_ids=list(range(NCORES)),
        trace=_trace, tmpdir=_tmpdir,
    )
    total = np.zeros((S, HID), np.float32)
    for r in res.results:
        total += r["out"].astype(np.float32)
    out = total.reshape(1, S, HID)
    if _trace:
        return out, res
    return out


# revision 58
# speedup vs baseline: 1.1500x; 1.1500x over previous
"""Trainium2 Bass kernel for RoPE + GQA causal attention (B=1, S=2048, HID=2048,
NH=16, NKV=4, HD=128), tensor-parallel over heads across 8 NeuronCores.

Core c computes q heads {2c, 2c+1} and kv head c//2 plus the matching wo
input-dim slice; each core emits a partial [S, HID] (f16) output and the host
sums the 8 partials.

v2 design notes (vs the 268us baseline):
  - 1024-wide bf16 matmul streams everywhere (proj, scores, attnV, rope raw),
    paired-head PSUM tiles [128, 2, 512] spanning 2 banks.
  - softmax denominator off the PE: bf16 pairwise tree-sum of the exp tiles on
    DVE + one tiny ones-matmul per (chunk, head); reciprocal on DVE,
    partition-broadcast on Pool.
  - causal masking by multiplying the exp tile with a 0/1 triangle (DVE),
    zeroing fully-masked columns with Pool memsets; no PSUM mask adds.
  - exp: ONE activation per k-tile covering both heads; ACT runs Exp/Copy only
    (single activation table, no table reloads).
  - phase interleave: proj chunk 0 -> attn cols [0,1024) -> proj chunk 1 ->
    attn cols [1024,2048); out-projection units of chunk j interleave into the
    PE stream of later chunks so the PE never idles (HAM stays at 2.4 GHz).
  - V transposed to [s,d] tiles via DMA-transpose (no PE/psum involved).
  - output partial written as f16 (half the DMA), copies split ACT/DVE.
"""

import os
import sys
from contextlib import ExitStack

for _p in ("/opt/trn_rl_repo", "/root/.axon_site/_ro/trn_rl_repo"):
    if os.path.isdir(_p) and _p not in sys.path:
        sys.path.append(_p)

import ml_dtypes
import numpy as np

import concourse.bass as bass
import concourse.mybir as mybir
import concourse.tile as tile
from concourse import bacc, bass_utils

S, HID, NH, NKV, HD = 2048, 2048, 16, 4, 128
HH = HD // 2  # 64
NCORES = 8
QH = NH // NCORES  # 2 q heads per core
SCALE = float(1.0 / np.sqrt(HD))

F32 = mybir.dt.float32
BF16 = mybir.dt.bfloat16
F16 = mybir.dt.float16
F8 = mybir.dt.float8e4
NPBF = ml_dtypes.bfloat16
NPF8 = ml_dtypes.float8_e4m3fn

# q/k projections run in fp8 (DoubleRow); weights are pre-scaled by W8S on the
# host, so scores carry W8S^2 and the exp scale divides it back out
W8S = 32.0

NKC = HID // 128   # 16 contraction chunks
NK2 = HID // 256   # 8 double-row contraction chunks
SC1 = 1024         # phase-1 s-chunk
NC1 = S // SC1     # 2
# phase-2 attention chunks: (col0, width); tail kept short (last chunk 128)
ATTN_CHUNKS = [(0, 512), (512, 512), (1024, 512), (1536, 384), (1920, 128)]


def build_graph():
    nc = bacc.Bacc(trn_type="TRN2", enable_partition_id=False)

    xT = nc.dram_tensor("xt", [HID, S], BF16, kind="ExternalInput")
    x8d = nc.dram_tensor("x8", [HID, S], F8, kind="ExternalInput")
    wvT = nc.dram_tensor("wvt", [HID, HD], BF16, kind="ExternalInput")
    wqk8d = nc.dram_tensor("wqk8", [HID, 3 * HD], F8, kind="ExternalInput")
    woT = nc.dram_tensor("wot", [QH * HD, HID], BF16, kind="ExternalInput")
    c1d = nc.dram_tensor("c1", [HD, S], BF16, kind="ExternalInput")
    c2d = nc.dram_tensor("c2", [HD, S], BF16, kind="ExternalInput")
    r1d = nc.dram_tensor("r1t", [HD, HD], BF16, kind="ExternalInput")
    r2d = nc.dram_tensor("r2t", [HD, HD], BF16, kind="ExternalInput")
    outd = nc.dram_tensor("out", [S, HID], F16, kind="ExternalOutput")

    xT_t = xT.rearrange("(ko p) s -> p ko s", p=128)       # [128, 16, 2048]
    x8_t = x8d.rearrange("(k2 two p) s -> p k2 two s", p=128, two=2)
    wv_t = wvT.rearrange("(ko p) o -> p ko o", p=128)      # [128, 16, 128]
    wqk8_t = wqk8d.rearrange("(k2 two p) o -> p k2 two o", p=128, two=2)
    wo_t = woT.rearrange("(g p) h -> p g h", p=128)        # [128, 2, 2048]

    with tile.TileContext(nc) as tc, ExitStack() as ctx:
        # ---------- pools (all open for the whole kernel; phases interleave)
        consts = ctx.enter_context(tc.tile_pool(name="consts", bufs=1))
        persist = ctx.enter_context(tc.tile_pool(name="persist", bufs=1))
        xtp = ctx.enter_context(tc.tile_pool(name="xtp", bufs=1))
        x8p = ctx.enter_context(tc.tile_pool(name="x8p", bufs=1))
        rawp = ctx.enter_context(tc.tile_pool(name="rawp", bufs=4))
        t12p = ctx.enter_context(tc.tile_pool(name="t12p", bufs=2))
        ep = ctx.enter_context(tc.tile_pool(name="ep", bufs=16))
        trp = ctx.enter_context(tc.tile_pool(name="trp", bufs=6))
        obp = ctx.enter_context(tc.tile_pool(name="obp", bufs=6))
        dbp = ctx.enter_context(tc.tile_pool(name="dbp", bufs=2))
        # PSUM: 4 + 2 + 2 = 8 banks exactly
        p1 = ctx.enter_context(
            tc.tile_pool(name="p1", bufs=2, space="PSUM"))   # [128,2,512]f32 x2
        p2 = ctx.enter_context(
            tc.tile_pool(name="p2", bufs=1, space="PSUM"))   # [128,2,512]f32 x1
        p3 = ctx.enter_context(
            tc.tile_pool(name="p3", bufs=2, space="PSUM"))   # [128,512]f32 x2

        # ---------- persistent SBUF
        q_pair = persist.tile([128, QH, S], BF16, tag="q_pair")
        kT = persist.tile([128, S], BF16, tag="kT")
        v_sd = persist.tile([128, S // 128, HD], BF16, tag="v_sd")
        ao_pair = persist.tile([128, QH, S], BF16, tag="ao_pair")
        wv_sb = persist.tile([128, NKC, HD], BF16, tag="wv_sb")
        wqk8_sb = persist.tile([128, NK2, 2, 3 * HD], F8, tag="wqk8_sb")
        wo_sb = persist.tile([128, QH, HID], BF16, tag="wo_sb")
        c1_sb = persist.tile([128, S], BF16, tag="c1_sb")
        c2_sb = persist.tile([128, S], BF16, tag="c2_sb")

        r1_sb = consts.tile([128, 128], BF16)
        r2_sb = consts.tile([128, 128], BF16)
        ones_col = consts.tile([128, 1], BF16)
        nc.vector.memset(ones_col, 1.0)
        # 0/1 lower-triangle keep mask (keep where col >= row), both heads
        tri01 = consts.tile([128, QH, 128], BF16)
        nc.gpsimd.memset(tri01, 1.0)
        nc.gpsimd.affine_select(
            out=tri01,
            in_=tri01,
            compare_op=mybir.AluOpType.is_ge,
            fill=0.0,
            base=0,
            pattern=[[0, QH], [1, 128]],  # value = col
            channel_multiplier=-1,        # - row
        )

        # ---------- initial DMAs (kc-parity interleave so chunks land in order)
        xt0 = xtp.tile([128, NKC, SC1], BF16, tag="xt", name="xt0")
        x80 = x8p.tile([128, NK2, 2, SC1], F8, tag="x8", name="x80")
        for kc in range(NKC):
            eng = nc.sync if kc % 2 == 0 else nc.gpsimd
            eng.dma_start(wv_sb[:, kc, :], wv_t[:, kc, :])
            eng.dma_start(xt0[:, kc, :], xT_t[:, kc, 0:SC1])
            if kc % 2 == 1:
                kc2 = kc // 2
                eng2 = nc.gpsimd if kc % 4 == 1 else nc.sync
                eng2.dma_start(wqk8_sb[:, kc2, :, :], wqk8_t[:, kc2, :, :])
                eng2.dma_start(x80[:, kc2, :, :], x8_t[:, kc2, :, 0:SC1])
        nc.scalar.dma_start(r1_sb, r1d[:, :])
        nc.scalar.dma_start(r2_sb, r2d[:, :])
        nc.scalar.dma_start(c1_sb, c1d[:, :])
        nc.scalar.dma_start(c2_sb, c2d[:, :])
        for g in range(QH):
            nc.scalar.dma_start(wo_sb[:, g, :], wo_t[:, g, :])

        # ---------------- out-projection work queue --------------------
        # each unit: 2 accumulating matmuls + psum->f16 copy + DMA out
        op_queue = []
        op_flip = [0]

        def emit_op_unit():
            if not op_queue:
                return False
            st, hc = op_queue.pop(0)
            ssl = slice(st * 128, (st + 1) * 128)
            hsl = slice(hc * 512, (hc + 1) * 512)
            ps_o = p3.tile([128, 512], F32, tag="oc", name="ps_o")
            nc.tensor.matmul(ps_o, ao_pair[:, 0, ssl], wo_sb[:, 0, hsl],
                             start=True, stop=False)
            nc.tensor.matmul(ps_o, ao_pair[:, 1, ssl], wo_sb[:, 1, hsl],
                             start=False, stop=True)
            ob = obp.tile([128, 512], F16, tag="ob", name="ob")
            if op_flip[0] % 8 < 5:
                nc.scalar.copy(out=ob, in_=ps_o)
            else:
                nc.vector.tensor_copy(out=ob, in_=ps_o)
            nc.sync.dma_start(outd[ssl, hsl], ob)
            op_flip[0] += 1
            return True

        def queue_op_chunk(c0, w):
            for st in range(c0 // 128, (c0 + w) // 128):
                for hc in range(4):
                    op_queue.append((st, hc))

        # ---------------- phase 1: projections + RoPE ------------------
        def rope_half(t, j1, hh, ps_uw):
            """DVE combine for rope half hh of target t (0,1=q heads, 2=k)."""
            ps_u, ps_w = ps_uw
            base = j1 * SC1 + hh * 512
            csl = slice(base, base + 512)
            t1 = t12p.tile([128, 512], F32, tag="t1", name="t1")
            t2 = t12p.tile([128, 512], F32, tag="t2", name="t2")
            nc.vector.tensor_mul(out=t1, in0=ps_u, in1=c1_sb[:, csl])
            nc.vector.tensor_mul(out=t2, in0=ps_w, in1=c2_sb[:, csl])
            dst = q_pair[:, t, csl] if t < 2 else kT[:, csl]
            nc.vector.tensor_add(out=dst, in0=t1, in1=t2)

        def p1_chunk(j1, xt, xt8, fin_prev=None):
            # pending rope matmul sub-steps: (t, hh, raw)
            rope_subs = []
            sub_i = [0]

            def emit_rope_sub():
                if not rope_subs:
                    return
                t, hh, raw = rope_subs.pop(0)
                ps_uw = p2.tile([128, 2, 512], F32, tag="uw", name="ps_uw")
                sub_i[0] += 1
                nc.tensor.matmul(ps_uw[:, 0, :], r1_sb, raw[:, hh, :],
                                 start=True, stop=True)
                nc.tensor.matmul(ps_uw[:, 1, :], r2_sb, raw[:, hh, :],
                                 start=True, stop=True)
                rope_half(t, j1, hh, (ps_uw[:, 0, :], ps_uw[:, 1, :]))

            DR = mybir.MatmulPerfMode.DoubleRow

            def dr_mm(ps, kc2, osl, start, stop):
                """fp8 DoubleRow projection matmul pair (both s-halves)."""
                nc.tensor.matmul(
                    ps[:, 0, :], wqk8_sb[:, kc2, :, osl],
                    xt8[:, kc2, :, 0:512],
                    start=start, stop=stop, perf_mode=DR,
                )
                nc.tensor.matmul(
                    ps[:, 1, :], wqk8_sb[:, kc2, :, osl],
                    xt8[:, kc2, :, 512:SC1],
                    start=start, stop=stop, perf_mode=DR,
                )

            def hook_a(kc):
                if kc == 4 and fin_prev is not None:
                    fin_prev()
                elif kc in (4, 8, 12):
                    emit_op_unit()
                    emit_op_unit()

            # pass A: v (bf16, DMA-paced) + k (fp8 double-row) together
            pa = p1.tile([128, 2, 512], F32, tag="mm", name="pa")
            pb = p1.tile([128, 2, 512], F32, tag="mm", name="pb")
            for kc in range(NKC):
                hook_a(kc)
                nc.tensor.matmul(
                    pa[:, 0, :], wv_sb[:, kc, :], xt[:, kc, 0:512],
                    start=(kc == 0), stop=(kc == NKC - 1),
                )
                nc.tensor.matmul(
                    pa[:, 1, :], wv_sb[:, kc, :], xt[:, kc, 512:SC1],
                    start=(kc == 0), stop=(kc == NKC - 1),
                )
                if kc % 2 == 1:
                    kc2 = kc // 2
                    dr_mm(pb, kc2, slice(256, 384),
                          start=(kc2 == 0), stop=(kc2 == NK2 - 1))
            raw_v = rawp.tile([128, 2, 512], BF16, tag="raw", name="raw_v")
            nc.scalar.copy(out=raw_v, in_=pa)
            raw_k = rawp.tile([128, 2, 512], BF16, tag="raw", name="raw_k")
            nc.scalar.copy(out=raw_k, in_=pb)
            for b in range(SC1 // 128):
                st = j1 * (SC1 // 128) + b
                vr2 = raw_v[:, b // 4, (b % 4) * 128:(b % 4 + 1) * 128]
                eng = nc.sync if b % 2 == 0 else nc.scalar
                eng.dma_start_transpose(v_sd[:, st, :], vr2)
            rope_subs.append((2, 0, raw_k))
            rope_subs.append((2, 1, raw_k))
            # pass B: q0 + q1 fp8 double-row (k rope drains inside)
            pq0 = p1.tile([128, 2, 512], F32, tag="mm", name="pq0")
            pq1 = p1.tile([128, 2, 512], F32, tag="mm", name="pq1")
            for kc2 in range(NK2):
                if kc2 in (2, 4, 6):
                    if rope_subs:
                        emit_rope_sub()
                    else:
                        emit_op_unit()
                        emit_op_unit()
                dr_mm(pq0, kc2, slice(0, 128),
                      start=(kc2 == 0), stop=(kc2 == NK2 - 1))
                dr_mm(pq1, kc2, slice(128, 256),
                      start=(kc2 == 0), stop=(kc2 == NK2 - 1))
            raw_q0 = rawp.tile([128, 2, 512], BF16, tag="raw", name="raw_q0")
            nc.scalar.copy(out=raw_q0, in_=pq0)
            raw_q1 = rawp.tile([128, 2, 512], BF16, tag="raw", name="raw_q1")
            nc.scalar.copy(out=raw_q1, in_=pq1)
            rope_subs.append((0, 0, raw_q0))
            rope_subs.append((1, 0, raw_q1))
            rope_subs.append((0, 1, raw_q0))
            rope_subs.append((1, 1, raw_q1))
            while rope_subs:
                emit_rope_sub()
                emit_op_unit()
                emit_op_unit()
                emit_op_unit()

        # ---------------- phase 2: attention chunk ---------------------
        # Returns a finalize closure (den -> recip -> broadcast -> normalize
        # -> queue out-projection) that the CALLER emits later, from inside
        # the next PE-busy region, so chunk boundaries never stall the PE.
        def attn_chunk(c0, w, fin_prev=None, last=False):
            nk = (c0 + w) // 128
            # tree state: list of (level, tile) for the den pairwise sum
            tree = []

            def tree_add(entry):
                tree.append(entry)
                while len(tree) >= 2 and tree[-1][0] == tree[-2][0]:
                    l1, a = tree.pop()
                    _, b = tree.pop()
                    s_ = trp.tile([128, QH, 512], BF16, tag="tr", name="tsum")
                    with nc.allow_low_precision("bf16 den tree"):
                        nc.vector.tensor_add(
                            out=s_[:, :, :w], in0=a[:, :, :w], in1=b[:, :, :w])
                    tree.append((l1 + 1, s_))

            pend = []  # (k, v0, e) awaiting attnV
            st = {"ps_av": None}

            def emit_attnv():
                pk, pv0, pe = pend.pop(0)
                if st["ps_av"] is None:
                    st["ps_av"] = p2.tile([128, 2, 512], F32, tag="uw",
                                          name="ps_av")
                for h in range(QH):
                    nc.tensor.matmul(
                        st["ps_av"][:, h, pv0:w], v_sd[:, pk, :],
                        pe[:, h, pv0:w],
                        start=(pk == 0), stop=(pk == nk - 1),
                    )

            for k in range(nk):
                lo = 128 * k - c0  # diag block offset in chunk cols
                v0 = max(lo, 0)
                ps_s = p1.tile([128, 2, 512], F32, tag="mm", name="ps_s")
                for h in range(QH):
                    nc.tensor.matmul(
                        ps_s[:, h, v0:w], kT[:, k * 128:(k + 1) * 128],
                        q_pair[:, h, c0 + v0:c0 + w], start=True, stop=True,
                    )
                e = ep.tile([128, QH, 512], BF16, tag="e", name="e")
                nc.scalar.activation(
                    out=e[:, :, v0:w], in_=ps_s[:, :, v0:w],
                    func=mybir.ActivationFunctionType.Exp,
                    scale=SCALE / (W8S * W8S),
                )
                if v0 > 0:
                    nc.gpsimd.memset(e[:, :, 0:v0], 0.0)
                if lo > -128:  # diagonal tile: mask cols [lo, lo+128)
                    d0, d1 = max(lo, 0), min(lo + 128, w)
                    nc.vector.tensor_mul(
                        out=e[:, :, d0:d1], in0=e[:, :, d0:d1],
                        in1=tri01[:, :, d0 - lo:d1 - lo])
                tree_add((0, e))
                pend.append((k, v0, e))
                if k == 1 and fin_prev is not None:
                    fin_prev()
                if len(pend) > 3:
                    emit_attnv()
                # drain out-projection units, keeping 3 in reserve to cover
                # the next chunk-boundary normalize latency
                # in the last chunk there is no next boundary to reserve
                # for: drain so units don't strand into the tail flush
                if last:
                    emit_op_unit()
                    emit_op_unit()
                elif k < 3 or len(op_queue) > 6:
                    emit_op_unit()
            while pend:
                emit_attnv()

            # collapse leftover tree nodes (mixed levels)
            while len(tree) > 1:
                _, a = tree.pop()
                l2, b = tree.pop()
                s_ = trp.tile([128, QH, 512], BF16, tag="tr", name="tsum")
                with nc.allow_low_precision("bf16 den tree"):
                    nc.vector.tensor_add(
                        out=s_[:, :, :w], in0=a[:, :, :w], in1=b[:, :, :w])
                tree.append((l2 + 1, s_))
            acc = tree[0][1]
            ps_av = st["ps_av"]

            def fin():
                # denominator -> reciprocal -> broadcast -> normalize
                rd = dbp.tile([1, QH, 512], F32, tag="rd", name="rd")
                for h in range(QH):
                    psd = p3.tile([128, 512], F32, tag="oc", name="psd")
                    nc.tensor.matmul(psd[0:1, :w], ones_col, acc[:, h, :w],
                                     start=True, stop=True)
                    nc.vector.reciprocal_approx_fast(
                        out=rd[:, h, :w], in_=psd[0:1, :w])
                db = dbp.tile([128, QH, 512], F32, tag="db", name="db")
                nc.gpsimd.partition_broadcast(db[:, :, :w], rd[:1, :, :w])
                nc.vector.tensor_mul(
                    out=ao_pair[:, :, c0:c0 + w], in0=ps_av[:, :, :w],
                    in1=db[:, :, :w])
                queue_op_chunk(c0, w)

            return fin

        # ---------------- emission order -------------------------------
        p1_chunk(0, xt0, x80)
        # prefetch x chunk 1 (bf16 on the ACT hw-dge queue, fp8 on sync)
        xt1 = xtp.tile([128, NKC, SC1], BF16, tag="xt", name="xt1")
        for kc in range(NKC):
            nc.scalar.dma_start(xt1[:, kc, :], xT_t[:, kc, SC1:2 * SC1])
        x81 = x8p.tile([128, NK2, 2, SC1], F8, tag="x8", name="x81")
        for kc2 in range(NK2):
            nc.sync.dma_start(x81[:, kc2, :, :], x8_t[:, kc2, :, SC1:2 * SC1])
        fin0 = attn_chunk(0, 512)
        fin1 = attn_chunk(512, 512, fin_prev=fin0)
        p1_chunk(1, xt1, x81, fin_prev=fin1)
        fin2 = attn_chunk(1024, 512)
        fin3 = attn_chunk(1536, 384, fin_prev=fin2)
        fin4 = attn_chunk(1920, 128, fin_prev=fin3, last=True)
        fin4()
        while emit_op_unit():
            pass

    nc.finalize()
    return nc


def shard_inputs(x, cos, sin, wq, wk, wv, wo):
    x = np.asarray(x, np.float32).reshape(S, HID)
    cos = np.asarray(cos, np.float32)
    sin = np.asarray(sin, np.float32)
    wq = np.asarray(wq, np.float32)
    wk = np.asarray(wk, np.float32)
    wv = np.asarray(wv, np.float32)
    wo = np.asarray(wo, np.float32)

    xT = np.ascontiguousarray(x.T).astype(NPBF)

    cos_h, sin_h = cos[:, :HH].T, sin[:, :HH].T       # [64, S]
    c1 = np.ascontiguousarray(
        np.concatenate([cos_h, -sin_h], axis=0)).astype(NPBF)
    c2 = np.ascontiguousarray(
        np.concatenate([sin_h, cos_h], axis=0)).astype(NPBF)

    r1 = np.zeros((HD, HD), np.float32)
    for i in range(HH // 2):
        r1[2 * i, 2 * i + 1] = -1.0
        r1[2 * i + 1, 2 * i] = 1.0
    r1[HH:, :] = r1[:HH, :]
    r2 = np.zeros((HD, HD), np.float32)
    for d in range(HH):
        r2[d, d + HH] = 1.0
        r2[d + HH, d + HH] = 1.0
    r1t = np.ascontiguousarray(r1.T).astype(NPBF)  # lhsT for out = R1 @ rhs
    r2t = np.ascontiguousarray(r2.T).astype(NPBF)

    x8 = np.ascontiguousarray(x.T).astype(NPF8)

    in_maps = []
    for c in range(NCORES):
        h0 = QH * c
        kvh = h0 * NKV // NH
        wq_c = wq[h0 * HD:(h0 + QH) * HD, :]             # [256, HID]
        wk_c = wk[kvh * HD:(kvh + 1) * HD, :]            # [128, HID]
        wv_c = wv[kvh * HD:(kvh + 1) * HD, :]
        wvT_c = np.ascontiguousarray(wv_c.T).astype(NPBF)
        wqk8_c = np.ascontiguousarray(
            (np.concatenate([wq_c, wk_c], axis=0) * W8S).T).astype(NPF8)
        woT_c = np.ascontiguousarray(
            wo[:, h0 * HD:(h0 + QH) * HD].T).astype(NPBF)
        in_maps.append({
            "xt": xT,
            "x8": x8,
            "wvt": wvT_c,
            "wqk8": wqk8_c,
            "wot": woT_c,
            "c1": c1,
            "c2": c2,
            "r1t": r1t,
            "r2t": r2t,
        })
    return in_maps


_CACHED_NC = None


def kernel(x, cos, sin, wq, wk, wv, wo, _trace=False, _tmpdir=None):
    global _CACHED_NC
    in_maps = shard_inputs(x, cos, sin, wq, wk, wv, wo)
    if _CACHED_NC is None:
        _CACHED_NC = build_graph()
    nc = _CACHED_NC
    res = bass_utils.run_bass_kernel_spmd(
        nc, in_maps, core_ids=list(range(NCORES)),
        trace=_trace, tmpdir=_tmpdir,
    )
    total = np.zeros((S, HID), np.float32)
    for r in res.results:
        total += r["out"].astype(np.float32)
    out = total.reshape(1, S, HID)
    if _trace:
        return out, res
    return out


# revision 59
# speedup vs baseline: 1.2091x; 1.0514x over previous
"""Trainium2 Bass kernel for RoPE + GQA causal attention (B=1, S=2048, HID=2048,
NH=16, NKV=4, HD=128), tensor-parallel over heads across 8 NeuronCores.

Core c computes q heads {2c, 2c+1} and kv head c//2 plus the matching wo
input-dim slice; each core emits a partial [S, HID] (f16) output and the host
sums the 8 partials.

v2 design notes (vs the 268us baseline):
  - 1024-wide bf16 matmul streams everywhere (proj, scores, attnV, rope raw),
    paired-head PSUM tiles [128, 2, 512] spanning 2 banks.
  - softmax denominator off the PE: bf16 pairwise tree-sum of the exp tiles on
    DVE + one tiny ones-matmul per (chunk, head); reciprocal on DVE,
    partition-broadcast on Pool.
  - causal masking by multiplying the exp tile with a 0/1 triangle (DVE),
    zeroing fully-masked columns with Pool memsets; no PSUM mask adds.
  - exp: ONE activation per k-tile covering both heads; ACT runs Exp/Copy only
    (single activation table, no table reloads).
  - phase interleave: proj chunk 0 -> attn cols [0,1024) -> proj chunk 1 ->
    attn cols [1024,2048); out-projection units of chunk j interleave into the
    PE stream of later chunks so the PE never idles (HAM stays at 2.4 GHz).
  - V transposed to [s,d] tiles via DMA-transpose (no PE/psum involved).
  - output partial written as f16 (half the DMA), copies split ACT/DVE.
"""

import os
import sys
from contextlib import ExitStack

for _p in ("/opt/trn_rl_repo", "/root/.axon_site/_ro/trn_rl_repo"):
    if os.path.isdir(_p) and _p not in sys.path:
        sys.path.append(_p)

import ml_dtypes
import numpy as np

import concourse.bass as bass
import concourse.mybir as mybir
import concourse.tile as tile
from concourse import bacc, bass_utils

S, HID, NH, NKV, HD = 2048, 2048, 16, 4, 128
HH = HD // 2  # 64
NCORES = 8
QH = NH // NCORES  # 2 q heads per core
SCALE = float(1.0 / np.sqrt(HD))

F32 = mybir.dt.float32
BF16 = mybir.dt.bfloat16
F16 = mybir.dt.float16
F8 = mybir.dt.float8e4
NPBF = ml_dtypes.bfloat16
NPF8 = ml_dtypes.float8_e4m3fn

# q/k projections run in fp8 (DoubleRow); weights are pre-scaled by W8S on the
# host, so scores carry W8S^2 and the exp scale divides it back out
W8S = 32.0

NKC = HID // 128   # 16 contraction chunks
NK2 = HID // 256   # 8 double-row contraction chunks
SC1 = 1024         # phase-1 s-chunk
NC1 = S // SC1     # 2
# phase-2 attention chunks: (col0, width); tail kept short (last chunk 128)
ATTN_CHUNKS = [(0, 512), (512, 512), (1024, 512), (1536, 384), (1920, 128)]


def build_graph():
    nc = bacc.Bacc(trn_type="TRN2", enable_partition_id=False)

    xT = nc.dram_tensor("xt", [HID, S], BF16, kind="ExternalInput")
    x8d = nc.dram_tensor("x8", [HID, S], F8, kind="ExternalInput")
    wvT = nc.dram_tensor("wvt", [HID, HD], BF16, kind="ExternalInput")
    wqk8d = nc.dram_tensor("wqk8", [HID, 3 * HD], F8, kind="ExternalInput")
    woT = nc.dram_tensor("wot", [QH * HD, HID], BF16, kind="ExternalInput")
    c1d = nc.dram_tensor("c1", [HD, S], BF16, kind="ExternalInput")
    c2d = nc.dram_tensor("c2", [HD, S], BF16, kind="ExternalInput")
    r1d = nc.dram_tensor("r1t", [HD, HD], BF16, kind="ExternalInput")
    r2d = nc.dram_tensor("r2t", [HD, HD], BF16, kind="ExternalInput")
    outd = nc.dram_tensor("out", [S, HID], F16, kind="ExternalOutput")

    xT_t = xT.rearrange("(ko p) s -> p ko s", p=128)       # [128, 16, 2048]
    x8_t = x8d.rearrange("(k2 two p) s -> p k2 two s", p=128, two=2)
    wv_t = wvT.rearrange("(ko p) o -> p ko o", p=128)      # [128, 16, 128]
    wqk8_t = wqk8d.rearrange("(k2 two p) o -> p k2 two o", p=128, two=2)
    wo_t = woT.rearrange("(g p) h -> p g h", p=128)        # [128, 2, 2048]

    with tile.TileContext(nc) as tc, ExitStack() as ctx:
        # ---------- pools (all open for the whole kernel; phases interleave)
        consts = ctx.enter_context(tc.tile_pool(name="consts", bufs=1))
        persist = ctx.enter_context(tc.tile_pool(name="persist", bufs=1))
        xtp = ctx.enter_context(tc.tile_pool(name="xtp", bufs=1))
        x8p = ctx.enter_context(tc.tile_pool(name="x8p", bufs=1))
        rawp = ctx.enter_context(tc.tile_pool(name="rawp", bufs=4))
        t12p = ctx.enter_context(tc.tile_pool(name="t12p", bufs=2))
        ep = ctx.enter_context(tc.tile_pool(name="ep", bufs=16))
        trp = ctx.enter_context(tc.tile_pool(name="trp", bufs=6))
        obp = ctx.enter_context(tc.tile_pool(name="obp", bufs=6))
        dbp = ctx.enter_context(tc.tile_pool(name="dbp", bufs=2))
        # PSUM: 4 + 2 + 2 = 8 banks exactly
        p1 = ctx.enter_context(
            tc.tile_pool(name="p1", bufs=2, space="PSUM"))   # [128,2,512]f32 x2
        p2 = ctx.enter_context(
            tc.tile_pool(name="p2", bufs=1, space="PSUM"))   # [128,2,512]f32 x1
        p3 = ctx.enter_context(
            tc.tile_pool(name="p3", bufs=2, space="PSUM"))   # [128,512]f32 x2

        # ---------- persistent SBUF
        q_pair = persist.tile([128, QH, S], BF16, tag="q_pair")
        kT = persist.tile([128, S], BF16, tag="kT")
        v_sd = persist.tile([128, S // 128, HD], BF16, tag="v_sd")
        ao_pair = persist.tile([128, QH, S], BF16, tag="ao_pair")
        wv_sb = persist.tile([128, NKC, HD], BF16, tag="wv_sb")
        wqk8_sb = persist.tile([128, NK2, 2, 3 * HD], F8, tag="wqk8_sb")
        wo_sb = persist.tile([128, QH, HID], BF16, tag="wo_sb")
        c1_sb = persist.tile([128, S], BF16, tag="c1_sb")
        c2_sb = persist.tile([128, S], BF16, tag="c2_sb")

        r1_sb = consts.tile([128, 128], BF16)
        r2_sb = consts.tile([128, 128], BF16)
        ones_col = consts.tile([128, 1], BF16)
        nc.vector.memset(ones_col, 1.0)
        # 0/1 lower-triangle keep mask (keep where col >= row), both heads
        tri01 = consts.tile([128, QH, 128], BF16)
        nc.gpsimd.memset(tri01, 1.0)
        nc.gpsimd.affine_select(
            out=tri01,
            in_=tri01,
            compare_op=mybir.AluOpType.is_ge,
            fill=0.0,
            base=0,
            pattern=[[0, QH], [1, 128]],  # value = col
            channel_multiplier=-1,        # - row
        )

        # ---------- initial DMAs (kc-parity interleave so chunks land in order)
        xt0 = xtp.tile([128, NKC, SC1], BF16, tag="xt", name="xt0")
        x80 = x8p.tile([128, NK2, 2, SC1], F8, tag="x8", name="x80")
        for kc in range(NKC):
            eng = nc.sync if kc % 2 == 0 else nc.gpsimd
            eng.dma_start(wv_sb[:, kc, :], wv_t[:, kc, :])
            eng.dma_start(xt0[:, kc, :], xT_t[:, kc, 0:SC1])
            if kc % 2 == 1:
                kc2 = kc // 2
                eng2 = nc.gpsimd if kc % 4 == 1 else nc.sync
                eng2.dma_start(wqk8_sb[:, kc2, :, :], wqk8_t[:, kc2, :, :])
                eng2.dma_start(x80[:, kc2, :, :], x8_t[:, kc2, :, 0:SC1])
        nc.scalar.dma_start(r1_sb, r1d[:, :])
        nc.scalar.dma_start(r2_sb, r2d[:, :])
        nc.scalar.dma_start(c1_sb, c1d[:, :])
        nc.scalar.dma_start(c2_sb, c2d[:, :])
        for g in range(QH):
            nc.scalar.dma_start(wo_sb[:, g, :], wo_t[:, g, :])

        # ---------------- out-projection work queue --------------------
        # each unit: 2 accumulating matmuls + psum->f16 copy + DMA out
        op_queue = []
        op_flip = [0]

        def emit_op_unit(act_copy=False):
            if not op_queue:
                return False
            st, hc = op_queue.pop(0)
            ssl = slice(st * 128, (st + 1) * 128)
            hsl = slice(hc * 512, (hc + 1) * 512)
            ps_o = p3.tile([128, 512], F32, tag="oc", name="ps_o")
            nc.tensor.matmul(ps_o, ao_pair[:, 0, ssl], wo_sb[:, 0, hsl],
                             start=True, stop=False)
            nc.tensor.matmul(ps_o, ao_pair[:, 1, ssl], wo_sb[:, 1, hsl],
                             start=False, stop=True)
            ob = obp.tile([128, 512], F16, tag="ob", name="ob")
            if act_copy or op_flip[0] % 8 < 5:
                nc.scalar.copy(out=ob, in_=ps_o)
            else:
                nc.vector.tensor_copy(out=ob, in_=ps_o)
            nc.sync.dma_start(outd[ssl, hsl], ob)
            op_flip[0] += 1
            return True

        def queue_op_chunk(c0, w):
            for st in range(c0 // 128, (c0 + w) // 128):
                for hc in range(4):
                    op_queue.append((st, hc))

        # ---------------- phase 1: projections + RoPE ------------------
        def rope_half(t, j1, hh, ps_uw):
            """DVE combine for rope half hh of target t (0,1=q heads, 2=k)."""
            ps_u, ps_w = ps_uw
            base = j1 * SC1 + hh * 512
            csl = slice(base, base + 512)
            t1 = t12p.tile([128, 512], F32, tag="t1", name="t1")
            t2 = t12p.tile([128, 512], F32, tag="t2", name="t2")
            nc.vector.tensor_mul(out=t1, in0=ps_u, in1=c1_sb[:, csl])
            nc.vector.tensor_mul(out=t2, in0=ps_w, in1=c2_sb[:, csl])
            dst = q_pair[:, t, csl] if t < 2 else kT[:, csl]
            nc.vector.tensor_add(out=dst, in0=t1, in1=t2)

        def p1_chunk(j1, xt, xt8, fin_prev=None):
            # pending rope matmul sub-steps: (t, hh, raw)
            rope_subs = []
            sub_i = [0]

            def emit_rope_sub():
                if not rope_subs:
                    return
                t, hh, raw = rope_subs.pop(0)
                ps_uw = p2.tile([128, 2, 512], F32, tag="uw", name="ps_uw")
                sub_i[0] += 1
                nc.tensor.matmul(ps_uw[:, 0, :], r1_sb, raw[:, hh, :],
                                 start=True, stop=True)
                nc.tensor.matmul(ps_uw[:, 1, :], r2_sb, raw[:, hh, :],
                                 start=True, stop=True)
                rope_half(t, j1, hh, (ps_uw[:, 0, :], ps_uw[:, 1, :]))

            DR = mybir.MatmulPerfMode.DoubleRow

            def dr_mm(ps, kc2, osl, start, stop):
                """fp8 DoubleRow projection matmul pair (both s-halves)."""
                nc.tensor.matmul(
                    ps[:, 0, :], wqk8_sb[:, kc2, :, osl],
                    xt8[:, kc2, :, 0:512],
                    start=start, stop=stop, perf_mode=DR,
                )
                nc.tensor.matmul(
                    ps[:, 1, :], wqk8_sb[:, kc2, :, osl],
                    xt8[:, kc2, :, 512:SC1],
                    start=start, stop=stop, perf_mode=DR,
                )

            def hook_a(kc):
                if kc == 4 and fin_prev is not None:
                    fin_prev()
                elif kc in (4, 8, 12):
                    emit_op_unit()
                    emit_op_unit()

            # pass A: v (bf16, DMA-paced) + k (fp8 double-row) together
            pa = p1.tile([128, 2, 512], F32, tag="mm", name="pa")
            pb = p1.tile([128, 2, 512], F32, tag="mm", name="pb")
            for kc in range(NKC):
                hook_a(kc)
                nc.tensor.matmul(
                    pa[:, 0, :], wv_sb[:, kc, :], xt[:, kc, 0:512],
                    start=(kc == 0), stop=(kc == NKC - 1),
                )
                nc.tensor.matmul(
                    pa[:, 1, :], wv_sb[:, kc, :], xt[:, kc, 512:SC1],
                    start=(kc == 0), stop=(kc == NKC - 1),
                )
                if kc % 2 == 1:
                    kc2 = kc // 2
                    dr_mm(pb, kc2, slice(256, 384),
                          start=(kc2 == 0), stop=(kc2 == NK2 - 1))
            raw_v = rawp.tile([128, 2, 512], BF16, tag="raw", name="raw_v")
            nc.scalar.copy(out=raw_v, in_=pa)
            raw_k = rawp.tile([128, 2, 512], BF16, tag="raw", name="raw_k")
            nc.scalar.copy(out=raw_k, in_=pb)
            for b in range(SC1 // 128):
                st = j1 * (SC1 // 128) + b
                vr2 = raw_v[:, b // 4, (b % 4) * 128:(b % 4 + 1) * 128]
                eng = nc.sync if b % 2 == 0 else nc.scalar
                eng.dma_start_transpose(v_sd[:, st, :], vr2)
            rope_subs.append((2, 0, raw_k))
            rope_subs.append((2, 1, raw_k))
            # pass B: q0 + q1 fp8 double-row (k rope drains inside)
            pq0 = p1.tile([128, 2, 512], F32, tag="mm", name="pq0")
            pq1 = p1.tile([128, 2, 512], F32, tag="mm", name="pq1")
            for kc2 in range(NK2):
                if kc2 in (2, 4, 6):
                    if rope_subs:
                        emit_rope_sub()
                    else:
                        emit_op_unit()
                        emit_op_unit()
                dr_mm(pq0, kc2, slice(0, 128),
                      start=(kc2 == 0), stop=(kc2 == NK2 - 1))
                dr_mm(pq1, kc2, slice(128, 256),
                      start=(kc2 == 0), stop=(kc2 == NK2 - 1))
            raw_q0 = rawp.tile([128, 2, 512], BF16, tag="raw", name="raw_q0")
            nc.scalar.copy(out=raw_q0, in_=pq0)
            raw_q1 = rawp.tile([128, 2, 512], BF16, tag="raw", name="raw_q1")
            nc.scalar.copy(out=raw_q1, in_=pq1)
            rope_subs.append((0, 0, raw_q0))
            rope_subs.append((1, 0, raw_q1))
            rope_subs.append((0, 1, raw_q0))
            rope_subs.append((1, 1, raw_q1))
            # tail units' psum->f16 copies go to ACT: the DVE must stay
            # clear for the rope combines that gate the next chunk's scores
            while rope_subs:
                emit_rope_sub()
                emit_op_unit(act_copy=True)
                emit_op_unit(act_copy=True)
                emit_op_unit(act_copy=True)

        # ---------------- phase 2: attention chunk ---------------------
        # Returns a finalize closure (den -> recip -> broadcast -> normalize
        # -> queue out-projection) that the CALLER emits later, from inside
        # the next PE-busy region, so chunk boundaries never stall the PE.
        def attn_chunk(c0, w, fin_prev=None, last=False):
            nk = (c0 + w) // 128
            # tree state: list of (level, tile) for the den pairwise sum
            tree = []

            def tree_add(entry):
                tree.append(entry)
                while len(tree) >= 2 and tree[-1][0] == tree[-2][0]:
                    l1, a = tree.pop()
                    _, b = tree.pop()
                    s_ = trp.tile([128, QH, 512], BF16, tag="tr", name="tsum")
                    with nc.allow_low_precision("bf16 den tree"):
                        nc.vector.tensor_add(
                            out=s_[:, :, :w], in0=a[:, :, :w], in1=b[:, :, :w])
                    tree.append((l1 + 1, s_))

            pend = []  # (k, v0, e) awaiting attnV
            st = {"ps_av": None}

            def emit_attnv():
                pk, pv0, pe = pend.pop(0)
                if st["ps_av"] is None:
                    st["ps_av"] = p2.tile([128, 2, 512], F32, tag="uw",
                                          name="ps_av")
                for h in range(QH):
                    nc.tensor.matmul(
                        st["ps_av"][:, h, pv0:w], v_sd[:, pk, :],
                        pe[:, h, pv0:w],
                        start=(pk == 0), stop=(pk == nk - 1),
                    )

            for k in range(nk):
                lo = 128 * k - c0  # diag block offset in chunk cols
                v0 = max(lo, 0)
                ps_s = p1.tile([128, 2, 512], F32, tag="mm", name="ps_s")
                for h in range(QH):
                    nc.tensor.matmul(
                        ps_s[:, h, v0:w], kT[:, k * 128:(k + 1) * 128],
                        q_pair[:, h, c0 + v0:c0 + w], start=True, stop=True,
                    )
                e = ep.tile([128, QH, 512], BF16, tag="e", name="e")
                nc.scalar.activation(
                    out=e[:, :, v0:w], in_=ps_s[:, :, v0:w],
                    func=mybir.ActivationFunctionType.Exp,
                    scale=SCALE / (W8S * W8S),
                )
                if v0 > 0:
                    nc.gpsimd.memset(e[:, :, 0:v0], 0.0)
                if lo > -128:  # diagonal tile: mask cols [lo, lo+128)
                    d0, d1 = max(lo, 0), min(lo + 128, w)
                    nc.vector.tensor_mul(
                        out=e[:, :, d0:d1], in0=e[:, :, d0:d1],
                        in1=tri01[:, :, d0 - lo:d1 - lo])
                tree_add((0, e))
                pend.append((k, v0, e))
                if k == 1 and fin_prev is not None:
                    fin_prev()
                if len(pend) > 3:
                    emit_attnv()
                # drain out-projection units, keeping 3 in reserve to cover
                # the next chunk-boundary normalize latency
                # in the last chunk there is no next boundary to reserve
                # for: drain so units don't strand into the tail flush
                if last:
                    if len(op_queue) > 3:
                        emit_op_unit()
                elif k < 3 or len(op_queue) > 6:
                    emit_op_unit()
            while pend:
                emit_attnv()

            # collapse leftover tree nodes (mixed levels)
            while len(tree) > 1:
                _, a = tree.pop()
                l2, b = tree.pop()
                s_ = trp.tile([128, QH, 512], BF16, tag="tr", name="tsum")
                with nc.allow_low_precision("bf16 den tree"):
                    nc.vector.tensor_add(
                        out=s_[:, :, :w], in0=a[:, :, :w], in1=b[:, :, :w])
                tree.append((l2 + 1, s_))
            acc = tree[0][1]
            ps_av = st["ps_av"]

            def fin():
                # denominator -> reciprocal -> broadcast -> normalize
                rd = dbp.tile([1, QH, 512], F32, tag="rd", name="rd")
                for h in range(QH):
                    psd = p3.tile([128, 512], F32, tag="oc", name="psd")
                    nc.tensor.matmul(psd[0:1, :w], ones_col, acc[:, h, :w],
                                     start=True, stop=True)
                    nc.vector.reciprocal_approx_fast(
                        out=rd[:, h, :w], in_=psd[0:1, :w])
                db = dbp.tile([128, QH, 512], F32, tag="db", name="db")
                nc.gpsimd.partition_broadcast(db[:, :, :w], rd[:1, :, :w])
                nc.vector.tensor_mul(
                    out=ao_pair[:, :, c0:c0 + w], in0=ps_av[:, :, :w],
                    in1=db[:, :, :w])
                queue_op_chunk(c0, w)

            return fin

        # ---------------- emission order -------------------------------
        p1_chunk(0, xt0, x80)
        # prefetch x chunk 1 (bf16 on the ACT hw-dge queue, fp8 on sync)
        xt1 = xtp.tile([128, NKC, SC1], BF16, tag="xt", name="xt1")
        for kc in range(NKC):
            nc.scalar.dma_start(xt1[:, kc, :], xT_t[:, kc, SC1:2 * SC1])
        x81 = x8p.tile([128, NK2, 2, SC1], F8, tag="x8", name="x81")
        for kc2 in range(NK2):
            nc.sync.dma_start(x81[:, kc2, :, :], x8_t[:, kc2, :, SC1:2 * SC1])
        fin0 = attn_chunk(0, 512)
        fin1 = attn_chunk(512, 512, fin_prev=fin0)
        p1_chunk(1, xt1, x81, fin_prev=fin1)
        fin2 = attn_chunk(1024, 512)
        fin3 = attn_chunk(1536, 384, fin_prev=fin2)
        fin4 = attn_chunk(1920, 128, fin_prev=fin3, last=True)
        emit_op_unit()
        emit_op_unit()
        fin4()
        while emit_op_unit():
            pass

    nc.finalize()
    return nc


def shard_inputs(x, cos, sin, wq, wk, wv, wo):
    x = np.asarray(x, np.float32).reshape(S, HID)
    cos = np.asarray(cos, np.float32)
    sin = np.asarray(sin, np.float32)
    wq = np.asarray(wq, np.float32)
    wk = np.asarray(wk, np.float32)
    wv = np.asarray(wv, np.float32)
    wo = np.asarray(wo, np.float32)

    xT = np.ascontiguousarray(x.T).astype(NPBF)

    cos_h, sin_h = cos[:, :HH].T, sin[:, :HH].T       # [64, S]
    c1 = np.ascontiguousarray(
        np.concatenate([cos_h, -sin_h], axis=0)).astype(NPBF)
    c2 = np.ascontiguousarray(
        np.concatenate([sin_h, cos_h], axis=0)).astype(NPBF)

    r1 = np.zeros((HD, HD), np.float32)
    for i in range(HH // 2):
        r1[2 * i, 2 * i + 1] = -1.0
        r1[2 * i + 1, 2 * i] = 1.0
    r1[HH:, :] = r1[:HH, :]
    r2 = np.zeros((HD, HD), np.float32)
    for d in range(HH):
        r2[d, d + HH] = 1.0
        r2[d + HH, d + HH] = 1.0
    r1t = np.ascontiguousarray(r1.T).astype(NPBF)  # lhsT for out = R1 @ rhs
    r2t = np.ascontiguousarray(r2.T).astype(NPBF)

    x8 = np.ascontiguousarray(x.T).astype(NPF8)

    in_maps = []
    for c in range(NCORES):
        h0 = QH * c
        kvh = h0 * NKV // NH
        wq_c = wq[h0 * HD:(h0 + QH) * HD, :]             # [256, HID]
        wk_c = wk[kvh * HD:(kvh + 1) * HD, :]            # [128, HID]
        wv_c = wv[kvh * HD:(kvh + 1) * HD, :]
        wvT_c = np.ascontiguousarray(wv_c.T).astype(NPBF)
        wqk8_c = np.ascontiguousarray(
            (np.concatenate([wq_c, wk_c], axis=0) * W8S).T).astype(NPF8)
        woT_c = np.ascontiguousarray(
            wo[:, h0 * HD:(h0 + QH) * HD].T).astype(NPBF)
        in_maps.append({
            "xt": xT,
            "x8": x8,
            "wvt": wvT_c,
            "wqk8": wqk8_c,
            "wot": woT_c,
            "c1": c1,
            "c2": c2,
            "r1t": r1t,
            "r2t": r2t,
        })
    return in_maps


_CACHED_NC = None


def kernel(x, cos, sin, wq, wk, wv, wo, _trace=False, _tmpdir=None):
    global _CACHED_NC
    in_maps = shard_inputs(x, cos, sin, wq, wk, wv, wo)
    if _CACHED_NC is None:
        _CACHED_NC = build_graph()
    nc = _CACHED_NC
    res = bass_utils.run_bass_kernel_spmd(
        nc, in_maps, core_ids=list(range(NCORES)),
        trace=_trace, tmpdir=_tmpdir,
    )
    total = np.zeros((S, HID), np.float32)
    for r in res.results:
        total += r["out"].astype(np.float32)
    out = total.reshape(1, S, HID)
    if _trace:
        return out, res
    return out
